# revision 20
# baseline (speedup 1.0000x reference)
"""Multi-head self-attention Trainium2 kernel (8 NeuronCores, SPMD).

Problem: B=2, N=4096, D=512, H=8 heads of dim 64.
  qkv = x @ qkv_w.T + qkv_b ; per-head attention with softmax(QK^T/8) ;
  out = attn @ out_w.T + out_b

Sharding: 16 (batch, head) pairs -> 8 cores, each core owns one batch b and
one head-PAIR (2 adjacent heads = a 128-row slice of the qkv projections).
Each core computes the full attention for its 2 heads over all 4096 rows and
a partial output projection; the host sums the 4 per-batch partials and adds
the (folded) biases.

On-chip layout strategy: everything is computed with the contraction dim on
partitions so no transposes are ever needed:
  Q^T,K^T [128d, 4096]  <- lhsT=W^T tiles, rhs=x^T
  V       [4096, 128d]  (natural; lhsT=x^T tile, rhs=Wv^T) + fused ones column
  S^T = K^T-stationary matmul, 2 heads row-packed (K=64 each) in the PE array
  P^T = exp(S^T) on ScalarE straight out of PSUM (no max-subtraction: |S|<~3)
  O^T accum = (V|1)-stationary matmul over P^T; row 64 = softmax denominator
  normalize via reciprocal + PE outer-product broadcast; partial y^T = Wout^T
  slice-stationary matmul.
Scale 1/sqrt(64) and all biases are folded on the host (wq*=0.125 etc.).
"""

import os
import numpy as np
import ml_dtypes

B, N, D, H, HD = 2, 4096, 512, 8, 64
NCORES = 8
KT_TILES = 4      # D / 128 contraction tiles
JT = 32           # N / 128 key tiles
ICH = 8           # N / 512 query chunks
P = 128

# compute dtype mode: "bf16" (fast), "mixed" (fp32 scores, bf16 PV),
# "fp32" (all fp32)
MODE = os.environ.get("ATTN_KERNEL_MODE", "bf16")

_BUILD_CACHE = {}


def _np_dt(dt):
    import concourse.mybir as mybir
    return np.dtype(ml_dtypes.bfloat16) if dt == mybir.dt.bfloat16 else np.dtype(np.float32)


def _build(mode):
    """Build (and cache) the compiled Bass program for all cores (SPMD)."""
    if mode in _BUILD_CACHE:
        return _BUILD_CACHE[mode]

    import concourse.bacc as bacc
    import concourse.mybir as mybir
    import concourse.tile as tile
    from concourse.bass import _add_dep_helper
    from contextlib import ExitStack

    f32 = mybir.dt.float32
    bf16 = mybir.dt.bfloat16
    if mode == "bf16":
        dt_qk, dt_pv = bf16, bf16
    elif mode == "mixed":
        dt_qk, dt_pv = f32, bf16
    else:
        dt_qk, dt_pv = f32, f32

    Exp = mybir.ActivationFunctionType.Exp

    nc = bacc.Bacc(None, target_bir_lowering=False)
    xt_d = nc.dram_tensor("xt", [KT_TILES, P, N], dt_qk, kind="ExternalInput")
    wqt_d = nc.dram_tensor("wqt", [KT_TILES, P, P], dt_qk, kind="ExternalInput")
    wkt_d = nc.dram_tensor("wkt", [KT_TILES, P, P], dt_qk, kind="ExternalInput")
    wvt_d = nc.dram_tensor("wvt", [KT_TILES, P, P], dt_qk, kind="ExternalInput")
    wot_d = nc.dram_tensor("wot", [2, HD, D], dt_pv, kind="ExternalInput")
    bq_d = nc.dram_tensor("bq", [P, 1], f32, kind="ExternalInput")
    bk_d = nc.dram_tensor("bk", [P, 1], f32, kind="ExternalInput")
    yp_d = nc.dram_tensor("yp", [KT_TILES, P, N], f32, kind="ExternalOutput")

    def ics(i):
        return slice(i * 512, (i + 1) * 512)

    def jts(j):
        return slice(j * P, (j + 1) * P)

    def mts(m):
        return slice(m * P, (m + 1) * P)

    with tile.TileContext(nc) as tc, ExitStack() as ctx:
        const = ctx.enter_context(tc.tile_pool(name="const", bufs=1))
        sp = ctx.enter_context(tc.tile_pool(name="spool", bufs=2, space="PSUM"))
        op = ctx.enter_context(tc.tile_pool(name="opool", bufs=3, space="PSUM"))
        mp = ctx.enter_context(tc.tile_pool(name="mpool", bufs=1, space="PSUM"))
        pp = ctx.enter_context(tc.tile_pool(name="ppool", bufs=6))
        yep = ctx.enter_context(tc.tile_pool(name="yepool", bufs=3))
        rrp = ctx.enter_context(tc.tile_pool(name="rrpool", bufs=2))
        rbp = ctx.enter_context(tc.tile_pool(name="rbpool", bufs=2))

        xt = const.tile([P, KT_TILES, N], dt_qk, tag="xt")
        wqt = const.tile([P, KT_TILES, P], dt_qk, tag="wqt")
        wkt = const.tile([P, KT_TILES, P], dt_qk, tag="wkt")
        wvt = const.tile([P, KT_TILES, P], dt_qk, tag="wvt")
        for k in range(KT_TILES):
            nc.sync.dma_start(xt[:, k, :], xt_d[k])
            nc.gpsimd.dma_start(wqt[:, k, :], wqt_d[k])
            nc.gpsimd.dma_start(wkt[:, k, :], wkt_d[k])
            nc.scalar.dma_start(wvt[:, k, :], wvt_d[k])
        wot = const.tile([HD, 2, D], dt_pv, tag="wot")
        for h in range(2):
            nc.scalar.dma_start(wot[:, h, :], wot_d[h])
        bq = const.tile([P, 1], f32, tag="bq")
        bk = const.tile([P, 1], f32, tag="bk")
        nc.gpsimd.dma_start(bq[:], bq_d[:])
        nc.gpsimd.dma_start(bk[:], bk_d[:])

        QT = const.tile([P, N], dt_qk, tag="QT")
        KT = const.tile([P, N], dt_qk, tag="KT")
        Vp = const.tile([P, JT, 130], dt_pv, tag="Vp")
        OT0 = const.tile([HD, N], dt_pv, tag="OT0")
        OT1 = const.tile([HD, N], dt_pv, tag="OT1")
        ones = const.tile([65, HD], f32, tag="ones")
        nc.vector.memset(ones[64:65, :], 1.0)
        actwarm = const.tile([1, 1], f32, tag="actwarm")
        nc.vector.memset(actwarm[:], 0.0)
        nc.scalar.activation(actwarm[:], actwarm[:], Exp)
        nc.vector.memset(Vp[:, :, 64:65], 1.0)
        nc.vector.memset(Vp[:, :, 129:130], 1.0)

        # ---- projection units (emitted interleaved into the attention loop
        # so the PE prefix before the first exp is tiny) ----
        def qproj_unit(ic):
            # Q^T[:, ic] (uses the otherwise-idle mp psum bank)
            ps = mp.tile([P, 512], f32, tag="mp", name=f"qp_{ic}")
            for k in range(KT_TILES):
                nc.tensor.matmul(ps[:], wqt[:, k, :], xt[:, k, ics(ic)],
                                 start=(k == 0), stop=(k == KT_TILES - 1))
            nc.vector.tensor_scalar_add(QT[:, ics(ic)], ps[:], bq[:, 0:1])

        def qproj_quarter(ic, q, after=None):
            # one [128,128] N-slice of the Q^T projection (~0.5us PE quantum)
            qs = slice(ics(ic).start + 128 * q, ics(ic).start + 128 * (q + 1))
            ps = mp.tile([P, 128], f32, tag="mp", name=f"qq_{ic}_{q}")
            for k in range(KT_TILES):
                mm = nc.tensor.matmul(ps[:], wqt[:, k, :], xt[:, k, qs],
                                      start=(k == 0), stop=(k == KT_TILES - 1))
                if after is not None and k == 0:
                    _add_dep_helper(mm.ins, after.ins, sync=False,
                                    reason="defer qproj behind attention")
            nc.vector.tensor_scalar_add(QT[:, qs], ps[:], bq[:, 0:1])

        def kproj_quarter(jc, q, after=None):
            # one [128,128] N-slice of the K^T projection (~0.5us PE quantum)
            qs = slice(ics(jc).start + 128 * q, ics(jc).start + 128 * (q + 1))
            ps = mp.tile([P, 128], f32, tag="mp", name=f"kq_{jc}_{q}")
            for k in range(KT_TILES):
                mm = nc.tensor.matmul(ps[:], wkt[:, k, :], xt[:, k, qs],
                                      start=(k == 0), stop=(k == KT_TILES - 1))
                if after is not None and k == 0:
                    _add_dep_helper(mm.ins, after.ins, sync=False,
                                    reason="defer kproj behind attention")
            nc.vector.tensor_scalar_add(KT[:, qs], ps[:], bk[:, 0:1])

        def kproj_unit(jc):
            # K^T[:, jc*512:(jc+1)*512]
            ps = mp.tile([P, 512], f32, tag="mp", name=f"kp_{jc}")
            for k in range(KT_TILES):
                nc.tensor.matmul(ps[:], wkt[:, k, :], xt[:, k, ics(jc)],
                                 start=(k == 0), stop=(k == KT_TILES - 1))
            nc.vector.tensor_scalar_add(KT[:, ics(jc)], ps[:], bk[:, 0:1])

        def vproj_unit(jt):
            # V[jt] (natural layout) + split into the two per-head Vp slabs
            ps = op.tile([P, P], f32, tag="o", name=f"vp_{jt}")
            for k in range(KT_TILES):
                nc.tensor.matmul(ps[:], xt[:, k, jts(jt)], wvt[:, k, :],
                                 start=(k == 0), stop=(k == KT_TILES - 1))
            nc.vector.tensor_copy(Vp[:, jt, 0:64], ps[:, 0:64])
            nc.vector.tensor_copy(Vp[:, jt, 65:129], ps[:, 64:128])

        AHEAD = 3  # how many j-tiles of K/V projection to keep ahead of use

        def kproj_unit_s(jc):
            # K^T chunk on an s-pool slot (prefix only: runs parallel to the
            # qproj on the mp bank)
            ps = sp.tile([P, 512], f32, tag="s", name=f"kps_{jc}")
            for k in range(KT_TILES):
                nc.tensor.matmul(ps[:], wkt[:, k, :], xt[:, k, ics(jc)],
                                 start=(k == 0), stop=(k == KT_TILES - 1))
            nc.vector.tensor_scalar_add(KT[:, ics(jc)], ps[:], bk[:, 0:1])

        # upfront: Q chunk 0 on mp, K chunks 0+1 on the two s-pool slots
        qproj_unit(0)
        kproj_unit_s(0)
        kproj_unit_s(1)

        # ---- attention (software-pipelined emission: S/exp of step t, PV of
        # step t-1, so the PE never queues a PV behind the exp it feeds).
        # finalize is staged: DVE-only work (psum evict + reciprocal) right
        # after the last PV; PE work (outer-product, OT mul, y projection)
        # several iterations later so the PE FIFO never waits on the slow
        # reciprocal. ----
        def finalize_a(ic, o0, o1):
            # PSUM evictions only -- frees the o banks fast; everything slow
            # happens later, off the PE critical path
            oss = []
            for i, o in enumerate((o0, o1)):
                os_ = rrp.tile([65, 512], f32, tag="os", name=f"os_{ic}_{i}")
                nc.vector.tensor_copy(os_[:], o[:])
                oss.append(os_)
            return oss

        def norm_quantum(ic, st, h, half, after=None):
            # normalize one head/half: OT[:, slice] = os[0:64] * (1/r) via PE
            # outer-product of the raw denominator (no recip dep in PE FIFO)
            # then reciprocal+mul on DVE.  ~0.5us of PE work per quantum.
            os_ = st[h]
            OTt = (OT0, OT1)[h]
            ls = slice(256 * half, 256 * (half + 1))
            hs = slice(ics(ic).start + 256 * half, ics(ic).start + 256 * (half + 1))
            pool_h = mp if h == 0 else op
            rb = pool_h.tile([HD, 256], f32, tag="mp" if h == 0 else "o",
                             name=f"rb_{ic}_{h}_{half}")
            mm = nc.tensor.matmul(rb[:], ones[64:65, :], os_[64:65, ls],
                                  start=True, stop=True, tile_position=(64, 0))
            if after is not None:
                _add_dep_helper(mm.ins, after.ins, sync=False,
                                reason="defer finalize rb behind attention")
            rbs = rbp.tile([HD, 256], f32, tag="rbs", name=f"rbs_{ic}_{h}_{half}")
            nc.vector.reciprocal(rbs[:], rb[:])
            nc.vector.tensor_mul(OTt[:, hs], os_[0:64, ls], rbs[:])

        def yproj_quantum(ic, mt, half, idx, after=None, tail=False):
            # one [128,256] slice of the partial output projection
            hs = slice(ics(ic).start + 256 * half, ics(ic).start + 256 * (half + 1))
            pool_y = mp if idx % 2 == 0 else op
            yps = pool_y.tile([P, 256], f32, tag="mp" if idx % 2 == 0 else "o",
                              name=f"yp_{ic}_{mt}_{half}")
            mm = nc.tensor.matmul(yps[:], wot[:, 0, mts(mt)], OT0[:, hs],
                                  start=True, stop=False)
            if after is not None:
                _add_dep_helper(mm.ins, after.ins, sync=False,
                                reason="defer finalize yproj behind attention")
            nc.tensor.matmul(yps[:], wot[:, 1, mts(mt)], OT1[:, hs],
                             start=False, stop=True)
            ye = yep.tile([P, 256], f32, tag="ye", name=f"ye_{ic}_{mt}_{half}")
            if tail:
                nc.scalar.copy(ye[:], yps[:])   # ScalarE is idle in the tail
            else:
                nc.vector.tensor_copy(ye[:], yps[:])
            nc.sync.dma_start(yp_d[mt, :, hs], ye[:])

        # quantum schedule within the NEXT chunk: (jt, fn(args))
        NORM_SCHED = [(2, (0, 0)), (4, (1, 0)), (5, (0, 1)), (7, (1, 1))]
        YP_JT0 = 9

        otiles = {}
        pend = None          # (p_tile, ic, jt) whose PV is not yet emitted
        pend_b = None        # (ic, stage-a state) awaiting finalize_b
        pend_c = None        # ic awaiting finalize_c
        for ic in range(ICH):
            otiles[ic] = (op.tile([65, 512], f32, tag="o", name=f"o0_{ic}"),
                          op.tile([65, 512], f32, tag="o", name=f"o1_{ic}"))
            for jt in range(JT):
                s = sp.tile([P, 1024], f32, tag="s")
                last_s = nc.tensor.matmul(s[:, 0:512], KT[0:64, jts(jt)],
                                          QT[0:64, ics(ic)],
                                          start=True, stop=True,
                                          tile_position=(0, 0))
                nc.tensor.matmul(s[:, 512:1024], KT[64:128, jts(jt)],
                                 QT[64:128, ics(ic)],
                                 start=True, stop=True, tile_position=(64, 0))
                p = pp.tile([P, 1024], dt_pv, tag="p")
                nc.scalar.activation(p[:], s[:], Exp)
                if pend is not None:
                    pp_, pic, pjt = pend
                    o0, o1 = otiles[pic]
                    nc.tensor.matmul(o0[:], Vp[:, pjt, 0:65], pp_[:, 0:512],
                                     start=(pjt == 0), stop=(pjt == JT - 1))
                    nc.tensor.matmul(o1[:], Vp[:, pjt, 65:130], pp_[:, 512:1024],
                                     start=(pjt == 0), stop=(pjt == JT - 1))
                    if pjt == JT - 1:
                        pend_b = (pic, finalize_a(pic, o0, o1))
                pend = (p, ic, jt)
                # deferred proj + finalize stages, spread across the loop
                if ic == 0:
                    if jt == 0:
                        vproj_unit(0)
                        vproj_unit(1)
                    elif jt <= JT - 2:
                        vproj_unit(jt + 1)
                    if jt < 24 and jt % 4 == 0:
                        kproj_unit(2 + jt // 4)
                if pend_b is not None:
                    bic, st = pend_b
                    for sjt, (h, half) in NORM_SCHED:
                        if jt == sjt:
                            norm_quantum(bic, st, h, half, after=last_s)
                    if YP_JT0 <= jt < YP_JT0 + 8:
                        idx = jt - YP_JT0        # A halves then B halves
                        half, mt = divmod(idx, 4)
                        yproj_quantum(bic, mt, half, idx, after=last_s)
                        if idx == 7:
                            pend_b = None
                if 24 <= jt < 28 and ic + 1 < ICH:
                    qproj_quarter(ic + 1, jt - 24, after=last_s)
        # drain the pipeline tail
        pp_, pic, pjt = pend
        o0, o1 = otiles[pic]
        nc.tensor.matmul(o0[:], Vp[:, pjt, 0:65], pp_[:, 0:512],
                         start=(pjt == 0), stop=(pjt == JT - 1))
        nc.tensor.matmul(o1[:], Vp[:, pjt, 65:130], pp_[:, 512:1024],
                         start=(pjt == 0), stop=(pjt == JT - 1))
        st = finalize_a(pic, o0, o1)
        for _, (h, half) in NORM_SCHED:
            norm_quantum(pic, st, h, half)
        for idx in range(8):
            half, mt = divmod(idx, 4)
            yproj_quantum(pic, mt, half, idx, tail=True)

    nc.compile()
    _BUILD_CACHE[mode] = nc
    return nc


def _prep_inputs(x, qkv_w, qkv_b, out_w, mode):
    """Per-core input maps. Core c: batch c//4, head-pair c%4."""
    if mode == "bf16":
        dt_qk = np.dtype(ml_dtypes.bfloat16)
        dt_pv = dt_qk
    elif mode == "mixed":
        dt_qk = np.dtype(np.float32)
        dt_pv = np.dtype(ml_dtypes.bfloat16)
    else:
        dt_qk = np.dtype(np.float32)
        dt_pv = dt_qk

    x = np.asarray(x, np.float32)
    qkv_w = np.asarray(qkv_w, np.float32)
    qkv_b = np.asarray(qkv_b, np.float32)
    out_w = np.asarray(out_w, np.float32)

    xts = []
    for b in range(B):
        xt = np.ascontiguousarray(x[b].T).reshape(KT_TILES, P, N)
        xts.append(xt.astype(dt_qk))

    in_maps = []
    for c in range(NCORES):
        b, m = divmod(c, 4)
        rs = slice(P * m, P * (m + 1))
        wq = (0.125 * qkv_w[0:D][rs]).T.reshape(KT_TILES, P, P)
        wk = qkv_w[D:2 * D][rs].T.reshape(KT_TILES, P, P)
        wv = qkv_w[2 * D:3 * D][rs].T.reshape(KT_TILES, P, P)
        wo = np.ascontiguousarray(out_w[:, rs].T).reshape(2, HD, D)
        in_maps.append({
            "xt": xts[b],
            "wqt": np.ascontiguousarray(wq).astype(dt_qk),
            "wkt": np.ascontiguousarray(wk).astype(dt_qk),
            "wvt": np.ascontiguousarray(wv).astype(dt_qk),
            "wot": wo.astype(dt_pv),
            "bq": (0.125 * qkv_b[0:D][rs]).reshape(P, 1).astype(np.float32),
            "bk": qkv_b[D:2 * D][rs].reshape(P, 1).astype(np.float32),
        })
    return in_maps


def _gather(results, qkv_b, out_w, out_b):
    # y[b] = (sum over the batch's 4 cores of yp)^T + out_w @ bv + out_b
    bias_vec = out_w.astype(np.float32) @ np.asarray(qkv_b, np.float32)[2 * D:3 * D] \
        + np.asarray(out_b, np.float32)
    y = np.empty((B, N, D), np.float32)
    for b in range(B):
        acc = np.zeros((D, N), np.float32)
        for m in range(4):
            acc += results[4 * b + m]["yp"].reshape(D, N)
        y[b] = acc.T + bias_vec
    return y


def _run(inputs, trace=False, tmpdir=None):
    from concourse.bass_utils import run_bass_kernel_spmd

    nc = _build(MODE)
    in_maps = _prep_inputs(inputs["x"], inputs["qkv_w"], inputs["qkv_b"],
                           inputs["out_w"], MODE)
    kw = {}
    if trace:
        kw = dict(trace=True, tmpdir=tmpdir)
    res = run_bass_kernel_spmd(nc, in_maps, core_ids=list(range(NCORES)), **kw)
    y = _gather(res.results, inputs["qkv_b"], inputs["out_w"], inputs["out_b"])
    return y, res


def kernel(x, qkv_w, qkv_b, out_w, out_b):
    y, _ = _run(dict(x=x, qkv_w=qkv_w, qkv_b=qkv_b, out_w=out_w, out_b=out_b))
    return y


# revision 21
# speedup vs baseline: 1.0319x; 1.0319x over previous
"""Multi-head self-attention Trainium2 kernel (8 NeuronCores, SPMD).

Problem: B=2, N=4096, D=512, H=8 heads of dim 64.
  qkv = x @ qkv_w.T + qkv_b ; per-head attention with softmax(QK^T/8) ;
  out = attn @ out_w.T + out_b

Sharding: 16 (batch, head) pairs -> 8 cores, each core owns one batch b and
one head-PAIR (2 adjacent heads = a 128-row slice of the qkv projections).
Each core computes the full attention for its 2 heads over all 4096 rows and
a partial output projection; the host sums the 4 per-batch partials and adds
the (folded) biases.

On-chip layout strategy: everything is computed with the contraction dim on
partitions so no transposes are ever needed:
  Q^T,K^T [128d, 4096]  <- lhsT=W^T tiles, rhs=x^T
  V       [4096, 128d]  (natural; lhsT=x^T tile, rhs=Wv^T) + fused ones column
  S^T = K^T-stationary matmul, 2 heads row-packed (K=64 each) in the PE array
  P^T = exp(S^T) on ScalarE straight out of PSUM (no max-subtraction: |S|<~3)
  O^T accum = (V|1)-stationary matmul over P^T; row 64 = softmax denominator
  normalize via reciprocal + PE outer-product broadcast; partial y^T = Wout^T
  slice-stationary matmul.
Scale 1/sqrt(64) and all biases are folded on the host (wq*=0.125 etc.).
"""

import os
import numpy as np
import ml_dtypes

B, N, D, H, HD = 2, 4096, 512, 8, 64
NCORES = 8
KT_TILES = 4      # D / 128 contraction tiles
JT = 32           # N / 128 key tiles
ICH = 8           # N / 512 query chunks
P = 128

# compute dtype mode: "bf16" (fast), "mixed" (fp32 scores, bf16 PV),
# "fp32" (all fp32)
MODE = os.environ.get("ATTN_KERNEL_MODE", "bf16")

_BUILD_CACHE = {}


def _np_dt(dt):
    import concourse.mybir as mybir
    return np.dtype(ml_dtypes.bfloat16) if dt == mybir.dt.bfloat16 else np.dtype(np.float32)


def _build(mode):
    """Build (and cache) the compiled Bass program for all cores (SPMD)."""
    if mode in _BUILD_CACHE:
        return _BUILD_CACHE[mode]

    import concourse.bacc as bacc
    import concourse.mybir as mybir
    import concourse.tile as tile
    from concourse.bass import _add_dep_helper
    from contextlib import ExitStack

    f32 = mybir.dt.float32
    bf16 = mybir.dt.bfloat16
    if mode == "bf16":
        dt_qk, dt_pv = bf16, bf16
    elif mode == "mixed":
        dt_qk, dt_pv = f32, bf16
    else:
        dt_qk, dt_pv = f32, f32

    Exp = mybir.ActivationFunctionType.Exp

    nc = bacc.Bacc(None, target_bir_lowering=False)
    xt_d = nc.dram_tensor("xt", [KT_TILES, P, N], dt_qk, kind="ExternalInput")
    wqt_d = nc.dram_tensor("wqt", [KT_TILES, P, P], dt_qk, kind="ExternalInput")
    wkt_d = nc.dram_tensor("wkt", [KT_TILES, P, P], dt_qk, kind="ExternalInput")
    wvt_d = nc.dram_tensor("wvt", [KT_TILES, P, P], dt_qk, kind="ExternalInput")
    wot_d = nc.dram_tensor("wot", [2, HD, D], dt_pv, kind="ExternalInput")
    bq_d = nc.dram_tensor("bq", [P, 1], f32, kind="ExternalInput")
    bk_d = nc.dram_tensor("bk", [P, 1], f32, kind="ExternalInput")
    yp_d = nc.dram_tensor("yp", [KT_TILES, P, N], f32, kind="ExternalOutput")

    def ics(i):
        return slice(i * 512, (i + 1) * 512)

    def jts(j):
        return slice(j * P, (j + 1) * P)

    def mts(m):
        return slice(m * P, (m + 1) * P)

    with tile.TileContext(nc) as tc, ExitStack() as ctx:
        const = ctx.enter_context(tc.tile_pool(name="const", bufs=1))
        sp = ctx.enter_context(tc.tile_pool(name="spool", bufs=2, space="PSUM"))
        op = ctx.enter_context(tc.tile_pool(name="opool", bufs=3, space="PSUM"))
        mp = ctx.enter_context(tc.tile_pool(name="mpool", bufs=1, space="PSUM"))
        pp = ctx.enter_context(tc.tile_pool(name="ppool", bufs=6))
        yep = ctx.enter_context(tc.tile_pool(name="yepool", bufs=3))
        rrp = ctx.enter_context(tc.tile_pool(name="rrpool", bufs=2))
        rbp = ctx.enter_context(tc.tile_pool(name="rbpool", bufs=2))

        xt = const.tile([P, KT_TILES, N], dt_qk, tag="xt")
        wqt = const.tile([P, KT_TILES, P], dt_qk, tag="wqt")
        wkt = const.tile([P, KT_TILES, P], dt_qk, tag="wkt")
        wvt = const.tile([P, KT_TILES, P], dt_qk, tag="wvt")
        for k in range(KT_TILES):
            nc.gpsimd.dma_start(wqt[:, k, :], wqt_d[k])
            nc.gpsimd.dma_start(wkt[:, k, :], wkt_d[k])
            nc.scalar.dma_start(wvt[:, k, :], wvt_d[k])
        # x^T in column-major chunk order on ONE queue: the first 1024-column
        # block of ALL k-tiles lands at ~25% of the transfer, so the Q/K
        # projections and early attention start ~10us sooner than waiting for
        # whole k-tiles (total landing time is HBM-stack-BW-bound either way)
        XCH = 1024
        for c in range(N // XCH):
            for k in range(KT_TILES):
                nc.sync.dma_start(xt[:, k, c * XCH:(c + 1) * XCH],
                                  xt_d[k][:, c * XCH:(c + 1) * XCH])
        wot = const.tile([HD, 2, D], dt_pv, tag="wot")
        for h in range(2):
            nc.scalar.dma_start(wot[:, h, :], wot_d[h])
        bq = const.tile([P, 1], f32, tag="bq")
        bk = const.tile([P, 1], f32, tag="bk")
        nc.gpsimd.dma_start(bq[:], bq_d[:])
        nc.gpsimd.dma_start(bk[:], bk_d[:])

        QT = const.tile([P, N], dt_qk, tag="QT")
        KT = const.tile([P, N], dt_qk, tag="KT")
        Vp = const.tile([P, JT, 130], dt_pv, tag="Vp")
        OT0 = const.tile([HD, N], dt_pv, tag="OT0")
        OT1 = const.tile([HD, N], dt_pv, tag="OT1")
        ones = const.tile([65, HD], f32, tag="ones")
        nc.vector.memset(ones[64:65, :], 1.0)
        actwarm = const.tile([1, 1], f32, tag="actwarm")
        nc.vector.memset(actwarm[:], 0.0)
        nc.scalar.activation(actwarm[:], actwarm[:], Exp)
        nc.vector.memset(Vp[:, :, 64:65], 1.0)
        nc.vector.memset(Vp[:, :, 129:130], 1.0)

        # ---- projection units (emitted interleaved into the attention loop
        # so the PE prefix before the first exp is tiny) ----
        def qproj_unit(ic):
            # Q^T[:, ic] (uses the otherwise-idle mp psum bank)
            ps = mp.tile([P, 512], f32, tag="mp", name=f"qp_{ic}")
            for k in range(KT_TILES):
                nc.tensor.matmul(ps[:], wqt[:, k, :], xt[:, k, ics(ic)],
                                 start=(k == 0), stop=(k == KT_TILES - 1))
            nc.vector.tensor_scalar_add(QT[:, ics(ic)], ps[:], bq[:, 0:1])

        def qproj_quarter(ic, q, after=None):
            # one [128,128] N-slice of the Q^T projection (~0.5us PE quantum)
            qs = slice(ics(ic).start + 128 * q, ics(ic).start + 128 * (q + 1))
            ps = mp.tile([P, 128], f32, tag="mp", name=f"qq_{ic}_{q}")
            for k in range(KT_TILES):
                mm = nc.tensor.matmul(ps[:], wqt[:, k, :], xt[:, k, qs],
                                      start=(k == 0), stop=(k == KT_TILES - 1))
                if after is not None and k == 0:
                    _add_dep_helper(mm.ins, after.ins, sync=False,
                                    reason="defer qproj behind attention")
            nc.vector.tensor_scalar_add(QT[:, qs], ps[:], bq[:, 0:1])

        def kproj_quarter(jc, q, after=None):
            # one [128,128] N-slice of the K^T projection (~0.5us PE quantum)
            qs = slice(ics(jc).start + 128 * q, ics(jc).start + 128 * (q + 1))
            ps = mp.tile([P, 128], f32, tag="mp", name=f"kq_{jc}_{q}")
            for k in range(KT_TILES):
                mm = nc.tensor.matmul(ps[:], wkt[:, k, :], xt[:, k, qs],
                                      start=(k == 0), stop=(k == KT_TILES - 1))
                if after is not None and k == 0:
                    _add_dep_helper(mm.ins, after.ins, sync=False,
                                    reason="defer kproj behind attention")
            nc.vector.tensor_scalar_add(KT[:, qs], ps[:], bk[:, 0:1])

        def kproj_unit(jc):
            # K^T[:, jc*512:(jc+1)*512]
            ps = mp.tile([P, 512], f32, tag="mp", name=f"kp_{jc}")
            for k in range(KT_TILES):
                nc.tensor.matmul(ps[:], wkt[:, k, :], xt[:, k, ics(jc)],
                                 start=(k == 0), stop=(k == KT_TILES - 1))
            nc.vector.tensor_scalar_add(KT[:, ics(jc)], ps[:], bk[:, 0:1])

        def vproj_unit(jt):
            # V[jt] (natural layout) + split into the two per-head Vp slabs
            ps = op.tile([P, P], f32, tag="o", name=f"vp_{jt}")
            for k in range(KT_TILES):
                nc.tensor.matmul(ps[:], xt[:, k, jts(jt)], wvt[:, k, :],
                                 start=(k == 0), stop=(k == KT_TILES - 1))
            nc.vector.tensor_copy(Vp[:, jt, 0:64], ps[:, 0:64])
            nc.vector.tensor_copy(Vp[:, jt, 65:129], ps[:, 64:128])

        AHEAD = 3  # how many j-tiles of K/V projection to keep ahead of use

        def kproj_unit_s(jc):
            # K^T chunk on an s-pool slot (prefix only: runs parallel to the
            # qproj on the mp bank)
            ps = sp.tile([P, 512], f32, tag="s", name=f"kps_{jc}")
            for k in range(KT_TILES):
                nc.tensor.matmul(ps[:], wkt[:, k, :], xt[:, k, ics(jc)],
                                 start=(k == 0), stop=(k == KT_TILES - 1))
            nc.vector.tensor_scalar_add(KT[:, ics(jc)], ps[:], bk[:, 0:1])

        # upfront: Q chunk 0 on mp, K chunks 0+1 on the two s-pool slots
        qproj_unit(0)
        kproj_unit_s(0)
        kproj_unit_s(1)

        # ---- attention (software-pipelined emission: S/exp of step t, PV of
        # step t-1, so the PE never queues a PV behind the exp it feeds).
        # finalize is staged: DVE-only work (psum evict + reciprocal) right
        # after the last PV; PE work (outer-product, OT mul, y projection)
        # several iterations later so the PE FIFO never waits on the slow
        # reciprocal. ----
        def finalize_a(ic, o0, o1):
            # PSUM evictions only -- frees the o banks fast; everything slow
            # happens later, off the PE critical path
            oss = []
            for i, o in enumerate((o0, o1)):
                os_ = rrp.tile([65, 512], f32, tag="os", name=f"os_{ic}_{i}")
                nc.vector.tensor_copy(os_[:], o[:])
                oss.append(os_)
            return oss

        def norm_quantum(ic, st, h, half, after=None):
            # normalize one head/half: OT[:, slice] = os[0:64] * (1/r) via PE
            # outer-product of the raw denominator (no recip dep in PE FIFO)
            # then reciprocal+mul on DVE.  ~0.5us of PE work per quantum.
            os_ = st[h]
            OTt = (OT0, OT1)[h]
            ls = slice(256 * half, 256 * (half + 1))
            hs = slice(ics(ic).start + 256 * half, ics(ic).start + 256 * (half + 1))
            pool_h = mp if h == 0 else op
            rb = pool_h.tile([HD, 256], f32, tag="mp" if h == 0 else "o",
                             name=f"rb_{ic}_{h}_{half}")
            mm = nc.tensor.matmul(rb[:], ones[64:65, :], os_[64:65, ls],
                                  start=True, stop=True, tile_position=(64, 0))
            if after is not None:
                _add_dep_helper(mm.ins, after.ins, sync=False,
                                reason="defer finalize rb behind attention")
            rbs = rbp.tile([HD, 256], f32, tag="rbs", name=f"rbs_{ic}_{h}_{half}")
            nc.vector.reciprocal(rbs[:], rb[:])
            nc.vector.tensor_mul(OTt[:, hs], os_[0:64, ls], rbs[:])

        def yproj_quantum(ic, mt, half, idx, after=None, tail=False):
            # one [128,256] slice of the partial output projection
            hs = slice(ics(ic).start + 256 * half, ics(ic).start + 256 * (half + 1))
            pool_y = mp if idx % 2 == 0 else op
            yps = pool_y.tile([P, 256], f32, tag="mp" if idx % 2 == 0 else "o",
                              name=f"yp_{ic}_{mt}_{half}")
            mm = nc.tensor.matmul(yps[:], wot[:, 0, mts(mt)], OT0[:, hs],
                                  start=True, stop=False)
            if after is not None:
                _add_dep_helper(mm.ins, after.ins, sync=False,
                                reason="defer finalize yproj behind attention")
            nc.tensor.matmul(yps[:], wot[:, 1, mts(mt)], OT1[:, hs],
                             start=False, stop=True)
            ye = yep.tile([P, 256], f32, tag="ye", name=f"ye_{ic}_{mt}_{half}")
            if tail:
                nc.scalar.copy(ye[:], yps[:])   # ScalarE is idle in the tail
            else:
                nc.vector.tensor_copy(ye[:], yps[:])
            nc.sync.dma_start(yp_d[mt, :, hs], ye[:])

        # quantum schedule within the NEXT chunk: (jt, fn(args))
        NORM_SCHED = [(2, (0, 0)), (4, (1, 0)), (5, (0, 1)), (7, (1, 1))]
        YP_JT0 = 9

        otiles = {}
        pend = None          # (p_tile, ic, jt) whose PV is not yet emitted
        pend_b = None        # (ic, stage-a state) awaiting finalize_b
        pend_c = None        # ic awaiting finalize_c
        for ic in range(ICH):
            otiles[ic] = (op.tile([65, 512], f32, tag="o", name=f"o0_{ic}"),
                          op.tile([65, 512], f32, tag="o", name=f"o1_{ic}"))
            for jt in range(JT):
                s = sp.tile([P, 1024], f32, tag="s")
                last_s = nc.tensor.matmul(s[:, 0:512], KT[0:64, jts(jt)],
                                          QT[0:64, ics(ic)],
                                          start=True, stop=True,
                                          tile_position=(0, 0))
                nc.tensor.matmul(s[:, 512:1024], KT[64:128, jts(jt)],
                                 QT[64:128, ics(ic)],
                                 start=True, stop=True, tile_position=(64, 0))
                p = pp.tile([P, 1024], dt_pv, tag="p")
                nc.scalar.activation(p[:], s[:], Exp)
                if pend is not None:
                    pp_, pic, pjt = pend
                    o0, o1 = otiles[pic]
                    nc.tensor.matmul(o0[:], Vp[:, pjt, 0:65], pp_[:, 0:512],
                                     start=(pjt == 0), stop=(pjt == JT - 1))
                    nc.tensor.matmul(o1[:], Vp[:, pjt, 65:130], pp_[:, 512:1024],
                                     start=(pjt == 0), stop=(pjt == JT - 1))
                    if pjt == JT - 1:
                        pend_b = (pic, finalize_a(pic, o0, o1))
                pend = (p, ic, jt)
                # deferred proj + finalize stages, spread across the loop
                if ic == 0:
                    if jt == 0:
                        vproj_unit(0)
                        vproj_unit(1)
                    elif jt <= JT - 2:
                        vproj_unit(jt + 1)
                    if jt < 24 and jt % 4 == 0:
                        kproj_unit(2 + jt // 4)
                if pend_b is not None:
                    bic, st = pend_b
                    for sjt, (h, half) in NORM_SCHED:
                        if jt == sjt:
                            norm_quantum(bic, st, h, half, after=last_s)
                    if YP_JT0 <= jt < YP_JT0 + 8:
                        idx = jt - YP_JT0        # A halves then B halves
                        half, mt = divmod(idx, 4)
                        yproj_quantum(bic, mt, half, idx, after=last_s)
                        if idx == 7:
                            pend_b = None
                if 24 <= jt < 28 and ic + 1 < ICH:
                    qproj_quarter(ic + 1, jt - 24, after=last_s)
        # drain the pipeline tail
        pp_, pic, pjt = pend
        o0, o1 = otiles[pic]
        nc.tensor.matmul(o0[:], Vp[:, pjt, 0:65], pp_[:, 0:512],
                         start=(pjt == 0), stop=(pjt == JT - 1))
        nc.tensor.matmul(o1[:], Vp[:, pjt, 65:130], pp_[:, 512:1024],
                         start=(pjt == 0), stop=(pjt == JT - 1))
        st = finalize_a(pic, o0, o1)
        for _, (h, half) in NORM_SCHED:
            norm_quantum(pic, st, h, half)
        for idx in range(8):
            half, mt = divmod(idx, 4)
            yproj_quantum(pic, mt, half, idx, tail=True)

    nc.compile()
    _BUILD_CACHE[mode] = nc
    return nc


def _prep_inputs(x, qkv_w, qkv_b, out_w, mode):
    """Per-core input maps. Core c: batch c//4, head-pair c%4."""
    if mode == "bf16":
        dt_qk = np.dtype(ml_dtypes.bfloat16)
        dt_pv = dt_qk
    elif mode == "mixed":
        dt_qk = np.dtype(np.float32)
        dt_pv = np.dtype(ml_dtypes.bfloat16)
    else:
        dt_qk = np.dtype(np.float32)
        dt_pv = dt_qk

    x = np.asarray(x, np.float32)
    qkv_w = np.asarray(qkv_w, np.float32)
    qkv_b = np.asarray(qkv_b, np.float32)
    out_w = np.asarray(out_w, np.float32)

    xts = []
    for b in range(B):
        xt = np.ascontiguousarray(x[b].T).reshape(KT_TILES, P, N)
        xts.append(xt.astype(dt_qk))

    in_maps = []
    for c in range(NCORES):
        b, m = divmod(c, 4)
        rs = slice(P * m, P * (m + 1))
        wq = (0.125 * qkv_w[0:D][rs]).T.reshape(KT_TILES, P, P)
        wk = qkv_w[D:2 * D][rs].T.reshape(KT_TILES, P, P)
        wv = qkv_w[2 * D:3 * D][rs].T.reshape(KT_TILES, P, P)
        wo = np.ascontiguousarray(out_w[:, rs].T).reshape(2, HD, D)
        in_maps.append({
            "xt": xts[b],
            "wqt": np.ascontiguousarray(wq).astype(dt_qk),
            "wkt": np.ascontiguousarray(wk).astype(dt_qk),
            "wvt": np.ascontiguousarray(wv).astype(dt_qk),
            "wot": wo.astype(dt_pv),
            "bq": (0.125 * qkv_b[0:D][rs]).reshape(P, 1).astype(np.float32),
            "bk": qkv_b[D:2 * D][rs].reshape(P, 1).astype(np.float32),
        })
    return in_maps


def _gather(results, qkv_b, out_w, out_b):
    # y[b] = (sum over the batch's 4 cores of yp)^T + out_w @ bv + out_b
    bias_vec = out_w.astype(np.float32) @ np.asarray(qkv_b, np.float32)[2 * D:3 * D] \
        + np.asarray(out_b, np.float32)
    y = np.empty((B, N, D), np.float32)
    for b in range(B):
        acc = np.zeros((D, N), np.float32)
        for m in range(4):
            acc += results[4 * b + m]["yp"].reshape(D, N)
        y[b] = acc.T + bias_vec
    return y


def _run(inputs, trace=False, tmpdir=None):
    from concourse.bass_utils import run_bass_kernel_spmd

    nc = _build(MODE)
    in_maps = _prep_inputs(inputs["x"], inputs["qkv_w"], inputs["qkv_b"],
                           inputs["out_w"], MODE)
    kw = {}
    if trace:
        kw = dict(trace=True, tmpdir=tmpdir)
    res = run_bass_kernel_spmd(nc, in_maps, core_ids=list(range(NCORES)), **kw)
    y = _gather(res.results, inputs["qkv_b"], inputs["out_w"], inputs["out_b"])
    return y, res


def kernel(x, qkv_w, qkv_b, out_w, out_b):
    y, _ = _run(dict(x=x, qkv_w=qkv_w, qkv_b=qkv_b, out_w=out_w, out_b=out_b))
    return y


# revision 22
# speedup vs baseline: 1.0340x; 1.0020x over previous
"""Multi-head self-attention Trainium2 kernel (8 NeuronCores, SPMD).

Problem: B=2, N=4096, D=512, H=8 heads of dim 64.
  qkv = x @ qkv_w.T + qkv_b ; per-head attention with softmax(QK^T/8) ;
  out = attn @ out_w.T + out_b

Sharding: 16 (batch, head) pairs -> 8 cores, each core owns one batch b and
one head-PAIR (2 adjacent heads = a 128-row slice of the qkv projections).
Each core computes the full attention for its 2 heads over all 4096 rows and
a partial output projection; the host sums the 4 per-batch partials and adds
the (folded) biases.

On-chip layout strategy: everything is computed with the contraction dim on
partitions so no transposes are ever needed:
  Q^T,K^T [128d, 4096]  <- lhsT=W^T tiles, rhs=x^T
  V       [4096, 128d]  (natural; lhsT=x^T tile, rhs=Wv^T) + fused ones column
  S^T = K^T-stationary matmul, 2 heads row-packed (K=64 each) in the PE array
  P^T = exp(S^T) on ScalarE straight out of PSUM (no max-subtraction: |S|<~3)
  O^T accum = (V|1)-stationary matmul over P^T; row 64 = softmax denominator
  normalize via reciprocal + PE outer-product broadcast; partial y^T = Wout^T
  slice-stationary matmul.
Scale 1/sqrt(64) and all biases are folded on the host (wq*=0.125 etc.).
"""

import os
import numpy as np
import ml_dtypes

B, N, D, H, HD = 2, 4096, 512, 8, 64
NCORES = 8
KT_TILES = 4      # D / 128 contraction tiles
JT = 32           # N / 128 key tiles
ICH = 8           # N / 512 query chunks
P = 128

# compute dtype mode: "bf16" (fast), "mixed" (fp32 scores, bf16 PV),
# "fp32" (all fp32)
MODE = os.environ.get("ATTN_KERNEL_MODE", "bf16")

_BUILD_CACHE = {}


def _np_dt(dt):
    import concourse.mybir as mybir
    return np.dtype(ml_dtypes.bfloat16) if dt == mybir.dt.bfloat16 else np.dtype(np.float32)


def _build(mode):
    """Build (and cache) the compiled Bass program for all cores (SPMD)."""
    if mode in _BUILD_CACHE:
        return _BUILD_CACHE[mode]

    import concourse.bacc as bacc
    import concourse.mybir as mybir
    import concourse.tile as tile
    from concourse.bass import _add_dep_helper
    from contextlib import ExitStack

    f32 = mybir.dt.float32
    bf16 = mybir.dt.bfloat16
    if mode == "bf16":
        dt_qk, dt_pv = bf16, bf16
    elif mode == "mixed":
        dt_qk, dt_pv = f32, bf16
    else:
        dt_qk, dt_pv = f32, f32

    Exp = mybir.ActivationFunctionType.Exp

    nc = bacc.Bacc(None, target_bir_lowering=False)
    xt_d = nc.dram_tensor("xt", [KT_TILES, P, N], dt_qk, kind="ExternalInput")
    wqt_d = nc.dram_tensor("wqt", [KT_TILES, P, P], dt_qk, kind="ExternalInput")
    wkt_d = nc.dram_tensor("wkt", [KT_TILES, P, P], dt_qk, kind="ExternalInput")
    wvt_d = nc.dram_tensor("wvt", [KT_TILES, P, P], dt_qk, kind="ExternalInput")
    wot_d = nc.dram_tensor("wot", [2, HD, D], dt_pv, kind="ExternalInput")
    bq_d = nc.dram_tensor("bq", [P, 1], f32, kind="ExternalInput")
    bk_d = nc.dram_tensor("bk", [P, 1], f32, kind="ExternalInput")
    yp_d = nc.dram_tensor("yp", [KT_TILES, P, N], f32, kind="ExternalOutput")

    def ics(i):
        return slice(i * 512, (i + 1) * 512)

    def jts(j):
        return slice(j * P, (j + 1) * P)

    def mts(m):
        return slice(m * P, (m + 1) * P)

    with tile.TileContext(nc) as tc, ExitStack() as ctx:
        const = ctx.enter_context(tc.tile_pool(name="const", bufs=1))
        sp = ctx.enter_context(tc.tile_pool(name="spool", bufs=2, space="PSUM"))
        op = ctx.enter_context(tc.tile_pool(name="opool", bufs=3, space="PSUM"))
        mp = ctx.enter_context(tc.tile_pool(name="mpool", bufs=1, space="PSUM"))
        pp = ctx.enter_context(tc.tile_pool(name="ppool", bufs=6))
        yep = ctx.enter_context(tc.tile_pool(name="yepool", bufs=3))
        rrp = ctx.enter_context(tc.tile_pool(name="rrpool", bufs=2))
        rbp = ctx.enter_context(tc.tile_pool(name="rbpool", bufs=2))

        xt = const.tile([P, KT_TILES, N], dt_qk, tag="xt")
        wqt = const.tile([P, KT_TILES, P], dt_qk, tag="wqt")
        wkt = const.tile([P, KT_TILES, P], dt_qk, tag="wkt")
        wvt = const.tile([P, KT_TILES, P], dt_qk, tag="wvt")
        for k in range(KT_TILES):
            nc.gpsimd.dma_start(wqt[:, k, :], wqt_d[k])
            nc.gpsimd.dma_start(wkt[:, k, :], wkt_d[k])
            nc.scalar.dma_start(wvt[:, k, :], wvt_d[k])
        # x^T in column-major chunk order on ONE queue: the first 1024-column
        # block of ALL k-tiles lands at ~25% of the transfer, so the Q/K
        # projections and early attention start ~10us sooner than waiting for
        # whole k-tiles (total landing time is HBM-stack-BW-bound either way)
        XCH = 512
        for c in range(N // XCH):
            for k in range(KT_TILES):
                nc.sync.dma_start(xt[:, k, c * XCH:(c + 1) * XCH],
                                  xt_d[k][:, c * XCH:(c + 1) * XCH])
        wot = const.tile([HD, 2, D], dt_pv, tag="wot")
        for h in range(2):
            nc.scalar.dma_start(wot[:, h, :], wot_d[h])
        bq = const.tile([P, 1], f32, tag="bq")
        bk = const.tile([P, 1], f32, tag="bk")
        nc.gpsimd.dma_start(bq[:], bq_d[:])
        nc.gpsimd.dma_start(bk[:], bk_d[:])

        QT = const.tile([P, N], dt_qk, tag="QT")
        KT = const.tile([P, N], dt_qk, tag="KT")
        Vp = const.tile([P, JT, 130], dt_pv, tag="Vp")
        OT0 = const.tile([HD, N], dt_pv, tag="OT0")
        OT1 = const.tile([HD, N], dt_pv, tag="OT1")
        ones = const.tile([65, HD], f32, tag="ones")
        nc.vector.memset(ones[64:65, :], 1.0)
        actwarm = const.tile([1, 1], f32, tag="actwarm")
        nc.vector.memset(actwarm[:], 0.0)
        nc.scalar.activation(actwarm[:], actwarm[:], Exp)
        nc.vector.memset(Vp[:, :, 64:65], 1.0)
        nc.vector.memset(Vp[:, :, 129:130], 1.0)

        # ---- projection units (emitted interleaved into the attention loop
        # so the PE prefix before the first exp is tiny) ----
        def qproj_unit(ic):
            # Q^T[:, ic] (uses the otherwise-idle mp psum bank)
            ps = mp.tile([P, 512], f32, tag="mp", name=f"qp_{ic}")
            for k in range(KT_TILES):
                nc.tensor.matmul(ps[:], wqt[:, k, :], xt[:, k, ics(ic)],
                                 start=(k == 0), stop=(k == KT_TILES - 1))
            nc.vector.tensor_scalar_add(QT[:, ics(ic)], ps[:], bq[:, 0:1])

        def qproj_quarter(ic, q, after=None):
            # one [128,128] N-slice of the Q^T projection (~0.5us PE quantum)
            qs = slice(ics(ic).start + 128 * q, ics(ic).start + 128 * (q + 1))
            ps = mp.tile([P, 128], f32, tag="mp", name=f"qq_{ic}_{q}")
            for k in range(KT_TILES):
                mm = nc.tensor.matmul(ps[:], wqt[:, k, :], xt[:, k, qs],
                                      start=(k == 0), stop=(k == KT_TILES - 1))
                if after is not None and k == 0:
                    _add_dep_helper(mm.ins, after.ins, sync=False,
                                    reason="defer qproj behind attention")
            nc.vector.tensor_scalar_add(QT[:, qs], ps[:], bq[:, 0:1])

        def kproj_quarter(jc, q, after=None):
            # one [128,128] N-slice of the K^T projection (~0.5us PE quantum)
            qs = slice(ics(jc).start + 128 * q, ics(jc).start + 128 * (q + 1))
            ps = mp.tile([P, 128], f32, tag="mp", name=f"kq_{jc}_{q}")
            for k in range(KT_TILES):
                mm = nc.tensor.matmul(ps[:], wkt[:, k, :], xt[:, k, qs],
                                      start=(k == 0), stop=(k == KT_TILES - 1))
                if after is not None and k == 0:
                    _add_dep_helper(mm.ins, after.ins, sync=False,
                                    reason="defer kproj behind attention")
            nc.vector.tensor_scalar_add(KT[:, qs], ps[:], bk[:, 0:1])

        def kproj_unit(jc):
            # K^T[:, jc*512:(jc+1)*512]
            ps = mp.tile([P, 512], f32, tag="mp", name=f"kp_{jc}")
            for k in range(KT_TILES):
                nc.tensor.matmul(ps[:], wkt[:, k, :], xt[:, k, ics(jc)],
                                 start=(k == 0), stop=(k == KT_TILES - 1))
            nc.vector.tensor_scalar_add(KT[:, ics(jc)], ps[:], bk[:, 0:1])

        def vproj_unit(jt):
            # V[jt] (natural layout) + split into the two per-head Vp slabs
            ps = op.tile([P, P], f32, tag="o", name=f"vp_{jt}")
            for k in range(KT_TILES):
                nc.tensor.matmul(ps[:], xt[:, k, jts(jt)], wvt[:, k, :],
                                 start=(k == 0), stop=(k == KT_TILES - 1))
            nc.vector.tensor_copy(Vp[:, jt, 0:64], ps[:, 0:64])
            nc.vector.tensor_copy(Vp[:, jt, 65:129], ps[:, 64:128])

        AHEAD = 3  # how many j-tiles of K/V projection to keep ahead of use

        def kproj_unit_s(jc):
            # K^T chunk on an s-pool slot (prefix only: runs parallel to the
            # qproj on the mp bank)
            ps = sp.tile([P, 512], f32, tag="s", name=f"kps_{jc}")
            for k in range(KT_TILES):
                nc.tensor.matmul(ps[:], wkt[:, k, :], xt[:, k, ics(jc)],
                                 start=(k == 0), stop=(k == KT_TILES - 1))
            nc.vector.tensor_scalar_add(KT[:, ics(jc)], ps[:], bk[:, 0:1])

        # upfront: Q chunk 0 on mp, K chunks 0+1 on the two s-pool slots
        qproj_unit(0)
        kproj_unit_s(0)
        kproj_unit_s(1)

        # ---- attention (software-pipelined emission: S/exp of step t, PV of
        # step t-1, so the PE never queues a PV behind the exp it feeds).
        # finalize is staged: DVE-only work (psum evict + reciprocal) right
        # after the last PV; PE work (outer-product, OT mul, y projection)
        # several iterations later so the PE FIFO never waits on the slow
        # reciprocal. ----
        def finalize_a(ic, o0, o1):
            # PSUM evictions only -- frees the o banks fast; everything slow
            # happens later, off the PE critical path
            oss = []
            for i, o in enumerate((o0, o1)):
                os_ = rrp.tile([65, 512], f32, tag="os", name=f"os_{ic}_{i}")
                nc.vector.tensor_copy(os_[:], o[:])
                oss.append(os_)
            return oss

        def norm_quantum(ic, st, h, half, after=None):
            # normalize one head/half: OT[:, slice] = os[0:64] * (1/r) via PE
            # outer-product of the raw denominator (no recip dep in PE FIFO)
            # then reciprocal+mul on DVE.  ~0.5us of PE work per quantum.
            os_ = st[h]
            OTt = (OT0, OT1)[h]
            ls = slice(256 * half, 256 * (half + 1))
            hs = slice(ics(ic).start + 256 * half, ics(ic).start + 256 * (half + 1))
            pool_h = mp if h == 0 else op
            rb = pool_h.tile([HD, 256], f32, tag="mp" if h == 0 else "o",
                             name=f"rb_{ic}_{h}_{half}")
            mm = nc.tensor.matmul(rb[:], ones[64:65, :], os_[64:65, ls],
                                  start=True, stop=True, tile_position=(64, 0))
            if after is not None:
                _add_dep_helper(mm.ins, after.ins, sync=False,
                                reason="defer finalize rb behind attention")
            rbs = rbp.tile([HD, 256], f32, tag="rbs", name=f"rbs_{ic}_{h}_{half}")
            nc.vector.reciprocal(rbs[:], rb[:])
            nc.vector.tensor_mul(OTt[:, hs], os_[0:64, ls], rbs[:])

        def yproj_quantum(ic, mt, half, idx, after=None, tail=False):
            # one [128,256] slice of the partial output projection
            hs = slice(ics(ic).start + 256 * half, ics(ic).start + 256 * (half + 1))
            pool_y = mp if idx % 2 == 0 else op
            yps = pool_y.tile([P, 256], f32, tag="mp" if idx % 2 == 0 else "o",
                              name=f"yp_{ic}_{mt}_{half}")
            mm = nc.tensor.matmul(yps[:], wot[:, 0, mts(mt)], OT0[:, hs],
                                  start=True, stop=False)
            if after is not None:
                _add_dep_helper(mm.ins, after.ins, sync=False,
                                reason="defer finalize yproj behind attention")
            nc.tensor.matmul(yps[:], wot[:, 1, mts(mt)], OT1[:, hs],
                             start=False, stop=True)
            ye = yep.tile([P, 256], f32, tag="ye", name=f"ye_{ic}_{mt}_{half}")
            if tail:
                nc.scalar.copy(ye[:], yps[:])   # ScalarE is idle in the tail
            else:
                nc.vector.tensor_copy(ye[:], yps[:])
            nc.sync.dma_start(yp_d[mt, :, hs], ye[:])

        # quantum schedule within the NEXT chunk: (jt, fn(args))
        NORM_SCHED = [(2, (0, 0)), (4, (1, 0)), (5, (0, 1)), (7, (1, 1))]
        YP_JT0 = 9

        otiles = {}
        pend = None          # (p_tile, ic, jt) whose PV is not yet emitted
        pend_b = None        # (ic, stage-a state) awaiting finalize_b
        pend_c = None        # ic awaiting finalize_c
        for ic in range(ICH):
            otiles[ic] = (op.tile([65, 512], f32, tag="o", name=f"o0_{ic}"),
                          op.tile([65, 512], f32, tag="o", name=f"o1_{ic}"))
            for jt in range(JT):
                s = sp.tile([P, 1024], f32, tag="s")
                last_s = nc.tensor.matmul(s[:, 0:512], KT[0:64, jts(jt)],
                                          QT[0:64, ics(ic)],
                                          start=True, stop=True,
                                          tile_position=(0, 0))
                nc.tensor.matmul(s[:, 512:1024], KT[64:128, jts(jt)],
                                 QT[64:128, ics(ic)],
                                 start=True, stop=True, tile_position=(64, 0))
                p = pp.tile([P, 1024], dt_pv, tag="p")
                nc.scalar.activation(p[:], s[:], Exp)
                if pend is not None:
                    pp_, pic, pjt = pend
                    o0, o1 = otiles[pic]
                    nc.tensor.matmul(o0[:], Vp[:, pjt, 0:65], pp_[:, 0:512],
                                     start=(pjt == 0), stop=(pjt == JT - 1))
                    nc.tensor.matmul(o1[:], Vp[:, pjt, 65:130], pp_[:, 512:1024],
                                     start=(pjt == 0), stop=(pjt == JT - 1))
                    if pjt == JT - 1:
                        pend_b = (pic, finalize_a(pic, o0, o1))
                pend = (p, ic, jt)
                # deferred proj + finalize stages, spread across the loop
                if ic == 0:
                    if jt == 0:
                        vproj_unit(0)
                        vproj_unit(1)
                    elif jt <= JT - 2:
                        vproj_unit(jt + 1)
                    if jt < 24 and jt % 4 == 0:
                        kproj_unit(2 + jt // 4)
                if pend_b is not None:
                    bic, st = pend_b
                    for sjt, (h, half) in NORM_SCHED:
                        if jt == sjt:
                            norm_quantum(bic, st, h, half, after=last_s)
                    if YP_JT0 <= jt < YP_JT0 + 8:
                        idx = jt - YP_JT0        # A halves then B halves
                        half, mt = divmod(idx, 4)
                        yproj_quantum(bic, mt, half, idx, after=last_s)
                        if idx == 7:
                            pend_b = None
                if 24 <= jt < 28 and ic + 1 < ICH:
                    qproj_quarter(ic + 1, jt - 24, after=last_s)
        # drain the pipeline tail
        pp_, pic, pjt = pend
        o0, o1 = otiles[pic]
        nc.tensor.matmul(o0[:], Vp[:, pjt, 0:65], pp_[:, 0:512],
                         start=(pjt == 0), stop=(pjt == JT - 1))
        nc.tensor.matmul(o1[:], Vp[:, pjt, 65:130], pp_[:, 512:1024],
                         start=(pjt == 0), stop=(pjt == JT - 1))
        st = finalize_a(pic, o0, o1)
        for _, (h, half) in NORM_SCHED:
            norm_quantum(pic, st, h, half)
        for idx in range(8):
            half, mt = divmod(idx, 4)
            yproj_quantum(pic, mt, half, idx, tail=True)

    nc.compile()
    _BUILD_CACHE[mode] = nc
    return nc


def _prep_inputs(x, qkv_w, qkv_b, out_w, mode):
    """Per-core input maps. Core c: batch c//4, head-pair c%4."""
    if mode == "bf16":
        dt_qk = np.dtype(ml_dtypes.bfloat16)
        dt_pv = dt_qk
    elif mode == "mixed":
        dt_qk = np.dtype(np.float32)
        dt_pv = np.dtype(ml_dtypes.bfloat16)
    else:
        dt_qk = np.dtype(np.float32)
        dt_pv = dt_qk

    x = np.asarray(x, np.float32)
    qkv_w = np.asarray(qkv_w, np.float32)
    qkv_b = np.asarray(qkv_b, np.float32)
    out_w = np.asarray(out_w, np.float32)

    xts = []
    for b in range(B):
        xt = np.ascontiguousarray(x[b].T).reshape(KT_TILES, P, N)
        xts.append(xt.astype(dt_qk))

    in_maps = []
    for c in range(NCORES):
        b, m = divmod(c, 4)
        rs = slice(P * m, P * (m + 1))
        wq = (0.125 * qkv_w[0:D][rs]).T.reshape(KT_TILES, P, P)
        wk = qkv_w[D:2 * D][rs].T.reshape(KT_TILES, P, P)
        wv = qkv_w[2 * D:3 * D][rs].T.reshape(KT_TILES, P, P)
        wo = np.ascontiguousarray(out_w[:, rs].T).reshape(2, HD, D)
        in_maps.append({
            "xt": xts[b],
            "wqt": np.ascontiguousarray(wq).astype(dt_qk),
            "wkt": np.ascontiguousarray(wk).astype(dt_qk),
            "wvt": np.ascontiguousarray(wv).astype(dt_qk),
            "wot": wo.astype(dt_pv),
            "bq": (0.125 * qkv_b[0:D][rs]).reshape(P, 1).astype(np.float32),
            "bk": qkv_b[D:2 * D][rs].reshape(P, 1).astype(np.float32),
        })
    return in_maps


def _gather(results, qkv_b, out_w, out_b):
    # y[b] = (sum over the batch's 4 cores of yp)^T + out_w @ bv + out_b
    bias_vec = out_w.astype(np.float32) @ np.asarray(qkv_b, np.float32)[2 * D:3 * D] \
        + np.asarray(out_b, np.float32)
    y = np.empty((B, N, D), np.float32)
    for b in range(B):
        acc = np.zeros((D, N), np.float32)
        for m in range(4):
            acc += results[4 * b + m]["yp"].reshape(D, N)
        y[b] = acc.T + bias_vec
    return y


def _run(inputs, trace=False, tmpdir=None):
    from concourse.bass_utils import run_bass_kernel_spmd

    nc = _build(MODE)
    in_maps = _prep_inputs(inputs["x"], inputs["qkv_w"], inputs["qkv_b"],
                           inputs["out_w"], MODE)
    kw = {}
    if trace:
        kw = dict(trace=True, tmpdir=tmpdir)
    res = run_bass_kernel_spmd(nc, in_maps, core_ids=list(range(NCORES)), **kw)
    y = _gather(res.results, inputs["qkv_b"], inputs["out_w"], inputs["out_b"])
    return y, res


def kernel(x, qkv_w, qkv_b, out_w, out_b):
    y, _ = _run(dict(x=x, qkv_w=qkv_w, qkv_b=qkv_b, out_w=out_w, out_b=out_b))
    return y
